# revision 36
# baseline (speedup 1.0000x reference)
"""Multi-head attention kernel for 8 TRN2 NeuronCores.

Problem: x[4,2048,1024] -> qkv proj (w_qkv[1024,3072]) -> 16-head attention
(dim_head=64, scale=1024**-0.5) -> out proj (w_out[1024,1024] + b_out).

Sharding: core c in 0..7 handles batch b=c//2, head-group g=c%2 (8 heads).
Each core computes a partial output y_partial = attn_out_g @ w_out[rows_g];
host sums the pair (the tensor-parallel all-reduce, done at unshard time).

Layout strategy (zero on-chip transposes):
  - host supplies xT = x[b].T                     [1024, 2048] fp16
  - qkT = (x @ w_qk).T computed directly:  lhsT=w chunk, rhs=xT  -> [c, i]
  - V   = x @ w_v computed normally:       lhsT=xT chunk, rhs=wv -> [i, c]
  - S^T = k_h @ q_h^T per head:            lhsT=kT slice, rhs=qT slice
          -> [keys, q]; heads processed in pairs, the even head in array
          row-group 0-63 and the odd head in 64-127, so their K=64
          matmuls run concurrently in the PE array
  - P   = exp(S^T * scale)  (no max subtraction: |S*scale| < ~1)
  - O^T|s = [v_h | 1]^T @ P : lhsT=v[128,65] (ones col), rhs=P -> [65, q]
          row 64 is the softmax denominator s
  - normalize off critical path: 1/s row bounced through DRAM and
    broadcast-DMA'd across partitions (no PSUM, no PE involvement)
  - y = sum_h (O_h^T).T @ w_out_h : lhsT=otn[64,128], rhs=wo -> [i, e]
The ScalarE exp stream (~280us) is the hard floor; the kernel keeps it
saturated: ST psum double-buffered, OT matmuls pipelined two kc behind
(PE queue is strict FIFO), normalize DVE/DMA work ordered so OT banks
free immediately, and warm-up/ramp dummy matmuls prevent the PE HAM
clock-gate from dropping to 1.2 GHz.
All matmul inputs fp16, PSUM accumulation fp32, output fp32.
"""

import numpy as np

B, N, D = 4, 2048, 1024
HEADS, DH = 16, 64
HP = HEADS // 2          # heads per core
GDIM = HP * DH           # 512 columns per head-group
SCALE = float(D) ** -0.5
NCORES = 8

_CACHE = {}


def _build():
    from contextlib import ExitStack

    import concourse.bass as bass
    import concourse.tile as tile
    from concourse import bacc, mybir

    F16 = mybir.dt.float16
    F32 = mybir.dt.float32
    EXP = mybir.ActivationFunctionType.Exp
    LN = mybir.ActivationFunctionType.Ln

    nc = bacc.Bacc(None, target_bir_lowering=False)

    xT_d = nc.declare_dram_parameter("xT", [D, N], F16, isOutput=False)
    wqk_d = nc.declare_dram_parameter("wqk", [D, 2 * GDIM], F16, isOutput=False)
    wv_d = nc.declare_dram_parameter("wv", [D, GDIM], F16, isOutput=False)
    wo_d = nc.declare_dram_parameter("wo", [4, 128, D], F16, isOutput=False)
    bias_d = nc.declare_dram_parameter("bias", [D], F32, isOutput=False)
    out_d = nc.declare_dram_parameter("out", [N, D], F32, isOutput=True)

    with tile.TileContext(nc) as tc, ExitStack() as ctx:
        persist = ctx.enter_context(tc.tile_pool(name="persist", bufs=1))
        ptp = ctx.enter_context(tc.tile_pool(name="ptp", bufs=7))
        rawp = ctx.enter_context(tc.tile_pool(name="rawp", bufs=5))
        tiny = ctx.enter_context(tc.tile_pool(name="tiny", bufs=4))
        ypool = ctx.enter_context(tc.tile_pool(name="ypool", bufs=2))
        dramp = ctx.enter_context(tc.tile_pool(name="dramp", bufs=4,
                                               space="DRAM"))
        # PSUM budget (8 banks): stq [128,1024] x2 bufs = 4, ot x4 = 4... see
        # tags: "stq" 2-bank tiles bufs=2, "ot0..3" 1 bank each, "qf" 1 bank
        mm = ctx.enter_context(tc.tile_pool(name="mm", bufs=2, space="PSUM"))
        acc = ctx.enter_context(tc.tile_pool(name="acc", bufs=1, space="PSUM"))

        # ---- persistent SBUF tiles -------------------------------------
        xT = [persist.tile([128, N], F16, name=f"xT{e}", tag=f"xT{e}")
              for e in range(8)]
        wqk = [persist.tile([128, 2 * GDIM], F16, name=f"wqk{e}", tag=f"wqk{e}")
               for e in range(8)]
        wv = [persist.tile([128, GDIM], F16, name=f"wv{e}", tag=f"wv{e}")
              for e in range(8)]
        wo = [persist.tile([128, D], F16, name=f"wo{tp}", tag=f"wo{tp}")
              for tp in range(4)]
        bias = persist.tile([128, D], F32, tag="bias")
        qkT = [persist.tile([128, N], F16, name=f"qkT{c}", tag=f"qkT{c}")
               for c in range(8)]
        vt = [persist.tile([128, HP, DH + 1], F16, name=f"v{kc}", tag=f"v{kc}")
              for kc in range(16)]
        otn = [persist.tile([128, N], F16, name=f"otn{tp}", tag=f"otn{tp}")
               for tp in range(4)]

        for e in range(8):
            nc.sync.dma_start(out=xT[e], in_=xT_d[e * 128:(e + 1) * 128, :])
            nc.sync.dma_start(out=wv[e], in_=wv_d[e * 128:(e + 1) * 128, :])
        for e in range(8):
            nc.sync.dma_start(out=wqk[e], in_=wqk_d[e * 128:(e + 1) * 128, :])
        for tp in range(4):
            nc.sync.dma_start(out=wo[tp], in_=wo_d[tp])
        bias_ap = bias_d[:]
        nc.sync.dma_start(
            out=bias,
            in_=bass.AP(tensor=bias_ap.tensor, offset=bias_ap.offset,
                        ap=[[0, 128]] + list(bias_ap.ap)),
        )
        for kc in range(16):
            nc.vector.memset(vt[kc][:, :, DH:DH + 1], 1.0)

        def qkv_chain(c, ih):
            """One [128,1024] qkT chunk: chunk c, query half ih."""
            ps = mm.tile([128, 1024], F32, name="stq", tag="stq")
            for e in range(8):
                w_sl = wqk[e][:, c * 128:(c + 1) * 128]
                yield nc.tensor.matmul(
                    ps[:, 0:512], lhsT=w_sl,
                    rhs=xT[e][:, ih * 1024:ih * 1024 + 512],
                    start=(e == 0), stop=(e == 7))
                yield nc.tensor.matmul(
                    ps[:, 512:1024], lhsT=w_sl,
                    rhs=xT[e][:, ih * 1024 + 512:(ih + 1) * 1024],
                    start=(e == 0), stop=(e == 7))
            yield nc.vector.tensor_copy(
                qkT[c][:, ih * 1024:(ih + 1) * 1024], ps)

        def v_chain(ih):
            """Two key-tiles of V via one [128,1024] psum tile."""
            ps = mm.tile([128, 1024], F32, name="stq", tag="stq")
            for e in range(8):
                yield nc.tensor.matmul(
                    ps[:, 0:512],
                    lhsT=xT[e][:, (2 * ih) * 128:(2 * ih + 1) * 128],
                    rhs=wv[e], start=(e == 0), stop=(e == 7))
                yield nc.tensor.matmul(
                    ps[:, 512:1024],
                    lhsT=xT[e][:, (2 * ih + 1) * 128:(2 * ih + 2) * 128],
                    rhs=wv[e], start=(e == 0), stop=(e == 7))
            for j in range(2):
                yield nc.vector.tensor_copy(
                    vt[2 * ih + j][:, :, 0:DH],
                    ps[:, j * 512:(j + 1) * 512].rearrange(
                        "p (h d) -> p h d", h=HP))

        # ---- PE warm-up: dummy matmuls during the input-DMA window ------
        wu = persist.tile([128, 512], F16, tag="wu")
        nc.vector.memset(wu, 0.0)
        wps = mm.tile([128, 1024], F32, name="stq", tag="stq")
        for r in range(32):
            nc.tensor.matmul(wps[:, 0:512], lhsT=wu[:, 0:128], rhs=wu,
                             start=True, stop=True)

        # ---- prelude: V first, then qkT ordered so pair-0 chunks land
        # last (dense PE hand-off into the first attention pass).
        # Chains round-robin over all 6 psum slots (2 stq + 4 ot tags) so
        # the psum->sbuf copies never stall the matmul stream.
        PSLOTS = ["ot0", "ot1", "qf0", "qf1"]

        def v_chain_small(it, slot):
            ps = acc.tile([128, 512], F32, name=f"pv{it}",
                          tag=PSLOTS[slot % 4])
            for e in range(8):
                yield nc.tensor.matmul(
                    ps, lhsT=xT[e][:, it * 128:(it + 1) * 128],
                    rhs=wv[e], start=(e == 0), stop=(e == 7))
            yield nc.vector.tensor_copy(
                vt[it][:, :, 0:DH],
                ps.rearrange("p (h d) -> p h d", h=HP))

        def qkv_chain_small(c, iq, slot):
            ps = acc.tile([128, 512], F32, name=f"pq{c}_{iq}",
                          tag=PSLOTS[slot % 4])
            for e in range(8):
                yield nc.tensor.matmul(
                    ps, lhsT=wqk[e][:, c * 128:(c + 1) * 128],
                    rhs=xT[e][:, iq * 512:(iq + 1) * 512],
                    start=(e == 0), stop=(e == 7))
            yield nc.vector.tensor_copy(
                qkT[c][:, iq * 512:(iq + 1) * 512], ps)

        gens = []
        for it in range(16):
            gens.append(("v", it))
        for c in (0, 4):
            for iq in range(4):
                gens.append(("qk", c, iq))
        # interleave: 2 big stq chains run as before; others on ot slots
        active = []
        gi = 0
        slot_rr = 0
        streams = []
        for g in gens:
            if g[0] == "v":
                streams.append(v_chain_small(g[1], slot_rr % 4))
            else:
                streams.append(qkv_chain_small(g[1], g[2], slot_rr % 4))
            slot_rr += 1
        # emit round-robin across 6 concurrent streams
        live = streams[:6]
        nxt = 6
        while live:
            done = []
            for s in live:
                if next(s, None) is None:
                    done.append(s)
            for s in done:
                live.remove(s)
                if nxt < len(streams):
                    live.append(streams[nxt])
                    nxt += 1

        # ---- attention: head pairs x q-quarters. Each stq tile holds both
        # heads' scores side by side ([A 512 | B 512]) so one FD=1024 exp
        # covers the pair, and each pass needs only TWO ot banks. The two
        # freed PSUM banks (qf0/qf1) host interleaved qkv filler chains
        # that hide most of the old prelude under the exp stream. ----------
        def emit_st_exp(t, qc, kc):
            qch, kch = t, 4 + t
            stq = mm.tile([128, 1024], F32, name="stq", tag="stq")
            nc.tensor.matmul(
                stq[:, 0:512],
                lhsT=qkT[kch][0:64, kc * 128:(kc + 1) * 128],
                rhs=qkT[qch][0:64, qc * 512:(qc + 1) * 512],
                start=True, stop=True)
            nc.tensor.matmul(
                stq[:, 512:1024],
                lhsT=qkT[kch][64:128, kc * 128:(kc + 1) * 128],
                rhs=qkT[qch][64:128, qc * 512:(qc + 1) * 512],
                start=True, stop=True)
            pt = ptp.tile([128, 1024], F16, name="pt", tag="pt")
            nc.scalar.activation(pt, stq, EXP, scale=SCALE)
            return pt

        # filler state: remaining qkT chunks as 9-item chains (8 MM + copy)
        fill_specs = []
        for tt in range(1, 4):
            for c in (tt, 4 + tt):
                for iq in range(4):
                    fill_specs.append((c, iq))

        def fill_chain(c, iq, slot):
            ps = acc.tile([128, 512], F32, name=f"qf{c}_{iq}",
                          tag=f"qf{slot}")
            for e in range(8):
                yield nc.tensor.matmul(
                    ps, lhsT=wqk[e][:, c * 128:(c + 1) * 128],
                    rhs=xT[e][:, iq * 512:(iq + 1) * 512],
                    start=(e == 0), stop=(e == 7))
            yield nc.vector.tensor_copy(
                qkT[c][:, iq * 512:(iq + 1) * 512], ps)

        fill_state = {"gen": None, "idx": 0, "slot": 0}

        def emit_fill(n, need_pair):
            """Emit up to n filler items, but only chains for pairs < need_pair
            deadline-wise; stops when specs are exhausted."""
            for _ in range(n):
                while True:
                    if fill_state["gen"] is None:
                        if fill_state["idx"] >= len(fill_specs):
                            return
                        c, iq = fill_specs[fill_state["idx"]]
                        fill_state["idx"] += 1
                        fill_state["slot"] ^= 1
                        fill_state["gen"] = fill_chain(c, iq,
                                                       fill_state["slot"])
                    if next(fill_state["gen"], None) is None:
                        fill_state["gen"] = None
                        continue
                    break

        passes = [(t, qc) for t in range(4) for qc in range(4)]
        hoisted = None
        for pi, (t, qc) in enumerate(passes):
            hA, hB = 2 * t, 2 * t + 1
            otA = acc.tile([65, 512], F32, name=f"otA{pi}", tag="ot0")
            otB = acc.tile([65, 512], F32, name=f"otB{pi}", tag="ot1")

            def emit_ot(kc, pt):
                st, sp = (kc == 0), (kc == 15)
                nc.tensor.matmul(otA, lhsT=vt[kc][:, hA, :],
                                 rhs=pt[:, 0:512], start=st, stop=sp,
                                 skip_group_check=True)
                nc.tensor.matmul(otB, lhsT=vt[kc][:, hB, :],
                                 rhs=pt[:, 512:1024], start=st, stop=sp,
                                 skip_group_check=True)

            pt_hist = []
            if hoisted is not None:
                for hk, hp in hoisted:
                    pt_hist.append((hk, hp))
                hoisted = None
                kc_start = 2
            else:
                kc_start = 0
            for kc in range(kc_start, 16):
                if pi == 0 and kc in (0, 1, 2):
                    for dj in (0, 1):
                        nc.tensor.matmul([otA, otB][dj], lhsT=wu[:, 0:65],
                                         rhs=wu, start=True, stop=True,
                                         skip_group_check=True)
                pt = emit_st_exp(t, qc, kc)
                pt_hist.append((kc, pt))
                if len(pt_hist) > 2:
                    k2, p2 = pt_hist.pop(0)
                    emit_ot(k2, p2)
                if t < 3:
                    emit_fill(1, t + 1)
            if t < 3:
                emit_fill(4, t + 1)
            if pi + 1 < len(passes):
                nt, nqc = passes[pi + 1]
                hoisted = [(0, emit_st_exp(nt, nqc, 0)),
                           (1, emit_st_exp(nt, nqc, 1))]
            for k2, p2 in pt_hist:
                emit_ot(k2, p2)

            # normalize the two heads (off critical path)
            last_pass = (pi == len(passes) - 1)
            raws, rcs, bcs = {}, {}, {}
            for j, (ott, hh) in enumerate(((otA, hA), (otB, hB))):
                raw = rawp.tile([65, 512], F16, name="raw", tag="raw")
                nc.vector.tensor_copy(raw, ott)
                raws[j] = raw
            for j in (0, 1):
                rc = tiny.tile([65, 512], F32, name="rc", tag="rc", bufs=4)
                if last_pass:
                    lntmp = tiny.tile([65, 512], F32, name="lntmp",
                                      tag="lntmp", bufs=2)
                    nc.scalar.activation(lntmp[64:65, :], raws[j][64:65, :],
                                         LN)
                    nc.scalar.activation(rc[64:65, :], lntmp[64:65, :],
                                         EXP, scale=-1.0)
                else:
                    with nc.allow_low_precision(reason="1/s fits f16"):
                        nc.vector.reciprocal(rc[64:65, :], raws[j][64:65, :])
                rcs[j] = rc
            shifted = {}
            for j in (0, 1):
                odd = (j == 1)
                dsc = dramp.tile([512], F32, name="dsc", tag="dsc")
                nc.sync.dma_start(out=dsc, in_=rcs[j][64:65, :])
                bc = tiny.tile([128, 512], F32, name="bc", tag="bc")
                dap = dsc[:]
                po = 64 if odd else 0
                nc.sync.dma_start(
                    out=bc[po:po + 64, :],
                    in_=bass.AP(tensor=dap.tensor, offset=dap.offset,
                                ap=[[0, 64]] + list(dap.ap)))
                bcs[j] = bc
                if odd:
                    rdsc = dramp.tile([64, 512], F16, name="rdsc",
                                      tag="rdsc", bufs=2)
                    nc.sync.dma_start(out=rdsc, in_=raws[j][0:64, :])
                    sh = rawp.tile([128, 512], F16, name="sh", tag="sh",
                                   bufs=2)
                    nc.sync.dma_start(out=sh[64:128, :], in_=rdsc[:])
                    shifted[j] = sh
            nc.vector.tensor_mul(
                otn[t][0:64, qc * 512:(qc + 1) * 512],
                raws[0][0:64, :], bcs[0][0:64, :])
            nc.vector.tensor_mul(
                otn[t][64:128, qc * 512:(qc + 1) * 512],
                shifted[1][64:128, :], bcs[1][64:128, :])

        # ---- output projection: 32 narrow chains, 6 psum slots ---------
        ptags = ["stq", "stq", "ot0", "ot1", "qf0", "qf1"]
        ppools = [mm, mm, acc, acc, acc, acc]
        ci = 0
        for it in range(16):
            for half in range(2):
                tag = ptags[ci % 6]
                ps = ppools[ci % 6].tile([128, 512], F32,
                                         name=f"pj{ci}", tag=tag)
                ci += 1
                e0 = half * 512
                for tp in range(4):
                    nc.tensor.matmul(
                        ps, lhsT=otn[tp][:, it * 128:(it + 1) * 128],
                        rhs=wo[tp][:, e0:e0 + 512],
                        start=(tp == 0), stop=(tp == 3))
                yt = ypool.tile([128, 512], F32, name="yt", tag="yt",
                                bufs=4)
                nc.vector.tensor_add(yt, ps, bias[:, e0:e0 + 512])
                yq = nc.sync if ci % 2 else nc.scalar
                yq.dma_start(
                    out=out_d[it * 128:(it + 1) * 128, e0:e0 + 512], in_=yt)

    nc.compile()
    return nc


def _in_maps(x, w_qkv, w_out, b_out):
    x = np.asarray(x, dtype=np.float32)
    w_qkv = np.asarray(w_qkv, dtype=np.float32)
    w_out = np.asarray(w_out, dtype=np.float32)
    b_out = np.asarray(b_out, dtype=np.float32)
    maps = []
    for c in range(NCORES):
        b, g = c // 2, c % 2
        qcols = w_qkv[:, g * GDIM:(g + 1) * GDIM]
        kcols = w_qkv[:, D + g * GDIM:D + (g + 1) * GDIM]
        vcols = w_qkv[:, 2 * D + g * GDIM:2 * D + (g + 1) * GDIM]
        maps.append({
            "xT": np.ascontiguousarray(x[b].T).astype(np.float16),
            "wqk": np.concatenate([qcols, kcols], axis=1).astype(np.float16),
            "wv": np.ascontiguousarray(vcols).astype(np.float16),
            "wo": np.ascontiguousarray(
                w_out[g * GDIM:(g + 1) * GDIM, :].reshape(4, 128, D)
            ).astype(np.float16),
            "bias": (b_out if g == 0 else np.zeros_like(b_out)),
        })
    return maps


def kernel(x, w_qkv, w_out, b_out):
    from concourse.bass_utils import run_bass_kernel_spmd

    if "nc" not in _CACHE:
        _CACHE["nc"] = _build()
    nc = _CACHE["nc"]
    maps = _in_maps(x, w_qkv, w_out, b_out)
    res = run_bass_kernel_spmd(nc, maps, core_ids=list(range(NCORES)))
    outs = res.results
    y = np.empty((B, N, D), dtype=np.float32)
    for b in range(B):
        y[b] = outs[2 * b]["out"] + outs[2 * b + 1]["out"]
    return y


# revision 37
# speedup vs baseline: 1.0528x; 1.0528x over previous
"""Multi-head attention kernel for 8 TRN2 NeuronCores.

Problem: x[4,2048,1024] -> qkv proj (w_qkv[1024,3072]) -> 16-head attention
(dim_head=64, scale=1024**-0.5) -> out proj (w_out[1024,1024] + b_out).

Sharding: core c in 0..7 handles batch b=c//2, head-group g=c%2 (8 heads).
Each core computes a partial output y_partial = attn_out_g @ w_out[rows_g];
host sums the pair (the tensor-parallel all-reduce, done at unshard time).

Layout strategy (zero on-chip transposes):
  - host supplies xT = x[b].T                     [1024, 2048] fp16
  - qkT = (x @ w_qk).T computed directly:  lhsT=w chunk, rhs=xT  -> [c, i]
  - V   = x @ w_v computed normally:       lhsT=xT chunk, rhs=wv -> [i, c]
  - S^T = k_h @ q_h^T per head:            lhsT=kT slice, rhs=qT slice
          -> [keys, q]; heads processed in pairs, the even head in array
          row-group 0-63 and the odd head in 64-127, so their K=64
          matmuls run concurrently in the PE array
  - P   = exp(S^T * scale)  (no max subtraction: |S*scale| < ~1)
  - O^T|s = [v_h | 1]^T @ P : lhsT=v[128,65] (ones col), rhs=P -> [65, q]
          row 64 is the softmax denominator s
  - normalize off critical path: 1/s row bounced through DRAM and
    broadcast-DMA'd across partitions (no PSUM, no PE involvement)
  - y = sum_h (O_h^T).T @ w_out_h : lhsT=otn[64,128], rhs=wo -> [i, e]
The ScalarE exp stream (~280us) is the hard floor; the kernel keeps it
saturated: ST psum double-buffered, OT matmuls pipelined two kc behind
(PE queue is strict FIFO), normalize DVE/DMA work ordered so OT banks
free immediately, and warm-up/ramp dummy matmuls prevent the PE HAM
clock-gate from dropping to 1.2 GHz.
All matmul inputs fp16, PSUM accumulation fp32, output fp32.
"""

import numpy as np

B, N, D = 4, 2048, 1024
HEADS, DH = 16, 64
HP = HEADS // 2          # heads per core
GDIM = HP * DH           # 512 columns per head-group
SCALE = float(D) ** -0.5
NCORES = 8

_CACHE = {}


def _build():
    from contextlib import ExitStack

    import concourse.bass as bass
    import concourse.tile as tile
    from concourse import bacc, mybir

    F16 = mybir.dt.float16
    F32 = mybir.dt.float32
    EXP = mybir.ActivationFunctionType.Exp
    LN = mybir.ActivationFunctionType.Ln

    nc = bacc.Bacc(None, target_bir_lowering=False)

    xT_d = nc.declare_dram_parameter("xT", [D, N], F16, isOutput=False)
    wqk_d = nc.declare_dram_parameter("wqk", [D, 2 * GDIM], F16, isOutput=False)
    wv_d = nc.declare_dram_parameter("wv", [D, GDIM], F16, isOutput=False)
    wo_d = nc.declare_dram_parameter("wo", [4, 128, D], F16, isOutput=False)
    bias_d = nc.declare_dram_parameter("bias", [D], F32, isOutput=False)
    out_d = nc.declare_dram_parameter("out", [N, D], F32, isOutput=True)

    with tile.TileContext(nc) as tc, ExitStack() as ctx:
        persist = ctx.enter_context(tc.tile_pool(name="persist", bufs=1))
        ptp = ctx.enter_context(tc.tile_pool(name="ptp", bufs=6))
        rawp = ctx.enter_context(tc.tile_pool(name="rawp", bufs=5))
        tiny = ctx.enter_context(tc.tile_pool(name="tiny", bufs=4))
        ypool = ctx.enter_context(tc.tile_pool(name="ypool", bufs=2))
        dramp = ctx.enter_context(tc.tile_pool(name="dramp", bufs=4,
                                               space="DRAM"))
        # PSUM budget (8 banks): stq [128,1024] x2 bufs = 4, ot x4 = 4... see
        # tags: "stq" 2-bank tiles bufs=2, "ot0..3" 1 bank each, "qf" 1 bank
        mm = ctx.enter_context(tc.tile_pool(name="mm", bufs=2, space="PSUM"))
        acc = ctx.enter_context(tc.tile_pool(name="acc", bufs=1, space="PSUM"))

        # ---- persistent SBUF tiles -------------------------------------
        xT = [persist.tile([128, N], F16, name=f"xT{e}", tag=f"xT{e}")
              for e in range(8)]
        wqk = [persist.tile([128, 2 * GDIM], F16, name=f"wqk{e}", tag=f"wqk{e}")
               for e in range(8)]
        wv = [persist.tile([128, GDIM], F16, name=f"wv{e}", tag=f"wv{e}")
              for e in range(8)]
        wo = [persist.tile([128, D], F16, name=f"wo{tp}", tag=f"wo{tp}")
              for tp in range(4)]
        bias = persist.tile([128, D], F32, tag="bias")
        qkT = [persist.tile([128, N], F16, name=f"qkT{c}", tag=f"qkT{c}")
               for c in range(8)]
        vt = [persist.tile([128, HP, DH + 1], F16, name=f"v{kc}", tag=f"v{kc}")
              for kc in range(16)]
        otn = [persist.tile([128, N], F16, name=f"otn{tp}", tag=f"otn{tp}")
               for tp in range(4)]

        for e in range(8):
            nc.sync.dma_start(out=xT[e], in_=xT_d[e * 128:(e + 1) * 128, :])
            nc.sync.dma_start(out=wv[e], in_=wv_d[e * 128:(e + 1) * 128, :])
        for e in range(8):
            nc.sync.dma_start(out=wqk[e], in_=wqk_d[e * 128:(e + 1) * 128, :])
        for tp in range(4):
            nc.sync.dma_start(out=wo[tp], in_=wo_d[tp])
        bias_ap = bias_d[:]
        nc.sync.dma_start(
            out=bias,
            in_=bass.AP(tensor=bias_ap.tensor, offset=bias_ap.offset,
                        ap=[[0, 128]] + list(bias_ap.ap)),
        )
        for kc in range(16):
            nc.vector.memset(vt[kc][:, :, DH:DH + 1], 1.0)

        def qkv_chain(c, ih):
            """One [128,1024] qkT chunk: chunk c, query half ih."""
            ps = mm.tile([128, 1024], F32, name="stq", tag="stq")
            for e in range(8):
                w_sl = wqk[e][:, c * 128:(c + 1) * 128]
                yield nc.tensor.matmul(
                    ps[:, 0:512], lhsT=w_sl,
                    rhs=xT[e][:, ih * 1024:ih * 1024 + 512],
                    start=(e == 0), stop=(e == 7))
                yield nc.tensor.matmul(
                    ps[:, 512:1024], lhsT=w_sl,
                    rhs=xT[e][:, ih * 1024 + 512:(ih + 1) * 1024],
                    start=(e == 0), stop=(e == 7))
            yield nc.vector.tensor_copy(
                qkT[c][:, ih * 1024:(ih + 1) * 1024], ps)

        def v_chain(ih):
            """Two key-tiles of V via one [128,1024] psum tile."""
            ps = mm.tile([128, 1024], F32, name="stq", tag="stq")
            for e in range(8):
                yield nc.tensor.matmul(
                    ps[:, 0:512],
                    lhsT=xT[e][:, (2 * ih) * 128:(2 * ih + 1) * 128],
                    rhs=wv[e], start=(e == 0), stop=(e == 7))
                yield nc.tensor.matmul(
                    ps[:, 512:1024],
                    lhsT=xT[e][:, (2 * ih + 1) * 128:(2 * ih + 2) * 128],
                    rhs=wv[e], start=(e == 0), stop=(e == 7))
            for j in range(2):
                yield nc.vector.tensor_copy(
                    vt[2 * ih + j][:, :, 0:DH],
                    ps[:, j * 512:(j + 1) * 512].rearrange(
                        "p (h d) -> p h d", h=HP))

        # ---- PE warm-up: dummy matmuls during the input-DMA window ------
        wu = persist.tile([128, 512], F16, tag="wu")
        nc.vector.memset(wu, 0.0)
        wps = mm.tile([128, 1024], F32, name="stq", tag="stq")
        for r in range(32):
            nc.tensor.matmul(wps[:, 0:512], lhsT=wu[:, 0:128], rhs=wu,
                             start=True, stop=True)

        # ---- prelude: V first, then qkT ordered so pair-0 chunks land
        # last (dense PE hand-off into the first attention pass).
        # Chains round-robin over all 6 psum slots (2 stq + 4 ot tags) so
        # the psum->sbuf copies never stall the matmul stream.
        PSLOTS = ["ot0", "ot1", "qf0", "qf1"]

        def v_chain_small(it, slot):
            ps = acc.tile([128, 512], F32, name=f"pv{it}",
                          tag=PSLOTS[slot % 4])
            for e in range(8):
                yield nc.tensor.matmul(
                    ps, lhsT=xT[e][:, it * 128:(it + 1) * 128],
                    rhs=wv[e], start=(e == 0), stop=(e == 7))
            yield nc.vector.tensor_copy(
                vt[it][:, :, 0:DH],
                ps.rearrange("p (h d) -> p h d", h=HP))

        def qkv_chain_small(c, iq, slot):
            ps = acc.tile([128, 512], F32, name=f"pq{c}_{iq}",
                          tag=PSLOTS[slot % 4])
            for e in range(8):
                yield nc.tensor.matmul(
                    ps, lhsT=wqk[e][:, c * 128:(c + 1) * 128],
                    rhs=xT[e][:, iq * 512:(iq + 1) * 512],
                    start=(e == 0), stop=(e == 7))
            yield nc.vector.tensor_copy(
                qkT[c][:, iq * 512:(iq + 1) * 512], ps)

        gens = []
        for it in range(16):
            gens.append(("v", it))
        for c in (0, 4):
            for iq in range(4):
                gens.append(("qk", c, iq))
        # interleave: 2 big stq chains run as before; others on ot slots
        active = []
        gi = 0
        slot_rr = 0
        streams = []
        for g in gens:
            if g[0] == "v":
                streams.append(v_chain_small(g[1], slot_rr % 4))
            else:
                streams.append(qkv_chain_small(g[1], g[2], slot_rr % 4))
            slot_rr += 1
        # emit round-robin across 6 concurrent streams
        live = streams[:6]
        nxt = 6
        while live:
            done = []
            for s in live:
                if next(s, None) is None:
                    done.append(s)
            for s in done:
                live.remove(s)
                if nxt < len(streams):
                    live.append(streams[nxt])
                    nxt += 1

        # ---- attention: head pairs x q-quarters. Each stq tile holds both
        # heads' scores side by side ([A 512 | B 512]) so one FD=1024 exp
        # covers the pair, and each pass needs only TWO ot banks. The two
        # freed PSUM banks (qf0/qf1) host interleaved qkv filler chains
        # that hide most of the old prelude under the exp stream. ----------
        def emit_st_exp(t, qc, kc):
            qch, kch = t, 4 + t
            stq = mm.tile([128, 1024], F32, name="stq", tag="stq")
            nc.tensor.matmul(
                stq[:, 0:512],
                lhsT=qkT[kch][0:64, kc * 128:(kc + 1) * 128],
                rhs=qkT[qch][0:64, qc * 512:(qc + 1) * 512],
                start=True, stop=True)
            nc.tensor.matmul(
                stq[:, 512:1024],
                lhsT=qkT[kch][64:128, kc * 128:(kc + 1) * 128],
                rhs=qkT[qch][64:128, qc * 512:(qc + 1) * 512],
                start=True, stop=True)
            pt = ptp.tile([128, 1024], F16, name="pt", tag="pt")
            nc.scalar.activation(pt, stq, EXP, scale=SCALE)
            return pt

        # filler state: remaining qkT chunks as 9-item chains (8 MM + copy)
        fill_specs = []
        for tt in range(1, 4):
            for c in (tt, 4 + tt):
                for iq in range(4):
                    fill_specs.append((c, iq))

        def fill_chain(c, iq, slot):
            ps = acc.tile([128, 512], F32, name=f"qf{c}_{iq}",
                          tag=f"qf{slot}")
            for e in range(8):
                yield nc.tensor.matmul(
                    ps, lhsT=wqk[e][:, c * 128:(c + 1) * 128],
                    rhs=xT[e][:, iq * 512:(iq + 1) * 512],
                    start=(e == 0), stop=(e == 7))
            yield nc.vector.tensor_copy(
                qkT[c][:, iq * 512:(iq + 1) * 512], ps)

        fill_state = {"gen": None, "idx": 0, "slot": 0}

        def emit_fill(n, need_pair):
            """Emit up to n filler items, but only chains for pairs < need_pair
            deadline-wise; stops when specs are exhausted."""
            for _ in range(n):
                while True:
                    if fill_state["gen"] is None:
                        if fill_state["idx"] >= len(fill_specs):
                            return
                        c, iq = fill_specs[fill_state["idx"]]
                        fill_state["idx"] += 1
                        fill_state["slot"] ^= 1
                        fill_state["gen"] = fill_chain(c, iq,
                                                       fill_state["slot"])
                    if next(fill_state["gen"], None) is None:
                        fill_state["gen"] = None
                        continue
                    break

        passes = [(t, qc) for t in range(4) for qc in range(4)]
        hoisted = None
        for pi, (t, qc) in enumerate(passes):
            hA, hB = 2 * t, 2 * t + 1
            otA = acc.tile([65, 512], F32, name=f"otA{pi}", tag="ot0")
            otB = acc.tile([65, 512], F32, name=f"otB{pi}", tag="ot1")

            def emit_ot(kc, pt):
                st, sp = (kc == 0), (kc == 15)
                nc.tensor.matmul(otA, lhsT=vt[kc][:, hA, :],
                                 rhs=pt[:, 0:512], start=st, stop=sp,
                                 skip_group_check=True)
                nc.tensor.matmul(otB, lhsT=vt[kc][:, hB, :],
                                 rhs=pt[:, 512:1024], start=st, stop=sp,
                                 skip_group_check=True)

            pt_hist = []
            if hoisted is not None:
                pt_hist.append((0, hoisted))
                hoisted = None
                kc_start = 1
            else:
                kc_start = 0
            for kc in range(kc_start, 16):
                if pi == 0 and kc in (0, 1, 2):
                    for dj in (0, 1):
                        nc.tensor.matmul([otA, otB][dj], lhsT=wu[:, 0:65],
                                         rhs=wu, start=True, stop=True,
                                         skip_group_check=True)
                pt = emit_st_exp(t, qc, kc)
                pt_hist.append((kc, pt))
                if len(pt_hist) > 2:
                    k2, p2 = pt_hist.pop(0)
                    emit_ot(k2, p2)
                if t < 3:
                    emit_fill(1, t + 1)
            if t < 3:
                emit_fill(4, t + 1)
            if pi + 1 < len(passes):
                nt, nqc = passes[pi + 1]
                hoisted = emit_st_exp(nt, nqc, 0)
            for k2, p2 in pt_hist:
                emit_ot(k2, p2)

            # normalize the two heads (off critical path)
            last_pass = (pi == len(passes) - 1)
            raws, rcs, bcs = {}, {}, {}
            for j, (ott, hh) in enumerate(((otA, hA), (otB, hB))):
                raw = rawp.tile([65, 512], F16, name="raw", tag="raw")
                nc.vector.tensor_copy(raw, ott)
                raws[j] = raw
            for j in (0, 1):
                rc = tiny.tile([65, 512], F32, name="rc", tag="rc", bufs=4)
                if last_pass:
                    lntmp = tiny.tile([65, 512], F32, name="lntmp",
                                      tag="lntmp", bufs=2)
                    nc.scalar.activation(lntmp[64:65, :], raws[j][64:65, :],
                                         LN)
                    nc.scalar.activation(rc[64:65, :], lntmp[64:65, :],
                                         EXP, scale=-1.0)
                else:
                    with nc.allow_low_precision(reason="1/s fits f16"):
                        nc.vector.reciprocal(rc[64:65, :], raws[j][64:65, :])
                rcs[j] = rc
            shifted = {}
            for j in (0, 1):
                odd = (j == 1)
                dsc = dramp.tile([512], F32, name="dsc", tag="dsc")
                nc.sync.dma_start(out=dsc, in_=rcs[j][64:65, :])
                bc = tiny.tile([128, 512], F32, name="bc", tag="bc")
                dap = dsc[:]
                po = 64 if odd else 0
                nc.sync.dma_start(
                    out=bc[po:po + 64, :],
                    in_=bass.AP(tensor=dap.tensor, offset=dap.offset,
                                ap=[[0, 64]] + list(dap.ap)))
                bcs[j] = bc
                if odd:
                    rdsc = dramp.tile([64, 512], F16, name="rdsc",
                                      tag="rdsc", bufs=2)
                    nc.sync.dma_start(out=rdsc, in_=raws[j][0:64, :])
                    sh = rawp.tile([128, 512], F16, name="sh", tag="sh",
                                   bufs=2)
                    nc.sync.dma_start(out=sh[64:128, :], in_=rdsc[:])
                    shifted[j] = sh
            nc.vector.tensor_mul(
                otn[t][0:64, qc * 512:(qc + 1) * 512],
                raws[0][0:64, :], bcs[0][0:64, :])
            nc.vector.tensor_mul(
                otn[t][64:128, qc * 512:(qc + 1) * 512],
                shifted[1][64:128, :], bcs[1][64:128, :])

        # ---- output projection: 32 narrow chains, 6 psum slots ---------
        ptags = ["stq", "stq", "ot0", "ot1", "qf0", "qf1"]
        ppools = [mm, mm, acc, acc, acc, acc]
        ci = 0
        for it in range(16):
            for half in range(2):
                tag = ptags[ci % 6]
                ps = ppools[ci % 6].tile([128, 512], F32,
                                         name=f"pj{ci}", tag=tag)
                ci += 1
                e0 = half * 512
                for tp in range(4):
                    nc.tensor.matmul(
                        ps, lhsT=otn[tp][:, it * 128:(it + 1) * 128],
                        rhs=wo[tp][:, e0:e0 + 512],
                        start=(tp == 0), stop=(tp == 3))
                yt = ypool.tile([128, 512], F32, name="yt", tag="yt",
                                bufs=4)
                nc.vector.tensor_add(yt, ps, bias[:, e0:e0 + 512])
                yq = nc.sync if ci % 2 else nc.scalar
                yq.dma_start(
                    out=out_d[it * 128:(it + 1) * 128, e0:e0 + 512], in_=yt)

    nc.compile()
    return nc


def _in_maps(x, w_qkv, w_out, b_out):
    x = np.asarray(x, dtype=np.float32)
    w_qkv = np.asarray(w_qkv, dtype=np.float32)
    w_out = np.asarray(w_out, dtype=np.float32)
    b_out = np.asarray(b_out, dtype=np.float32)
    maps = []
    for c in range(NCORES):
        b, g = c // 2, c % 2
        qcols = w_qkv[:, g * GDIM:(g + 1) * GDIM]
        kcols = w_qkv[:, D + g * GDIM:D + (g + 1) * GDIM]
        vcols = w_qkv[:, 2 * D + g * GDIM:2 * D + (g + 1) * GDIM]
        maps.append({
            "xT": np.ascontiguousarray(x[b].T).astype(np.float16),
            "wqk": np.concatenate([qcols, kcols], axis=1).astype(np.float16),
            "wv": np.ascontiguousarray(vcols).astype(np.float16),
            "wo": np.ascontiguousarray(
                w_out[g * GDIM:(g + 1) * GDIM, :].reshape(4, 128, D)
            ).astype(np.float16),
            "bias": (b_out if g == 0 else np.zeros_like(b_out)),
        })
    return maps


def kernel(x, w_qkv, w_out, b_out):
    from concourse.bass_utils import run_bass_kernel_spmd

    if "nc" not in _CACHE:
        _CACHE["nc"] = _build()
    nc = _CACHE["nc"]
    maps = _in_maps(x, w_qkv, w_out, b_out)
    res = run_bass_kernel_spmd(nc, maps, core_ids=list(range(NCORES)))
    outs = res.results
    y = np.empty((B, N, D), dtype=np.float32)
    for b in range(B):
        y[b] = outs[2 * b]["out"] + outs[2 * b + 1]["out"]
    return y
